# revision 29
# baseline (speedup 1.0000x reference)
"""Trainium2 Bass kernel for nn_CrossAttention_56092272886201.

Talking-heads cross-attention, b=2, n=m=2048, dim=64, heads=8, dh=dv=8.
Sharding: 8 cores = (batch 2) x (query-chunks of 512). Each core is fully
independent (talking-heads mixes the heads axis, which stays on-core; the
query axis i is sharded), so no collectives are needed.

Per-core layout (i-tile = 16 queries x 8 heads = 128 partitions):
  S[(h,i), j]     QK^T via block-diagonal packed q, fp32r matmuls (1 cy/row)
  E = exp(S)      ACT, fused row-sum -> den (per (h,i) partition)
  tw = WD/den     one GPSIMD normalize_recip op, bf16 out (den is per-head,
                  so the softmax normalization must precede the head mixing)
  A^T[j,(i,g)]    talk matmul, lhsT=E-chunk, rhs=tw
  Abf             ACT copy of A PSUM->SBUF bf16
  SQ = Abf^2      GPSIMD, m2 = sum_g SQ via DVE windowed tensor_reduce
  r               DVE-only rsqrt of v = m2/8 + eps: quake seed (int
                  shift/xor/add) + one tuned Newton step, no ACT table swaps
  P = Abf*r*C3    one DVE STT op (all-bf16), C3 fixes NR sign/scale
  out             P @ V_ln + beta via K=1 matmul; mask+reduce extracts h=g
W_talk is centered host-side over g so the heads-LayerNorm mean is exactly 0.
"""

import numpy as np

DIM = 64
HEADS = 8
N = 2048
B = 2
NCORES = 8
ICHUNK = 512          # queries per core
NT = 32               # i-tiles per core (16 queries each)
EPS = 1e-5

# quake-III rsqrt with one tuned Newton-Raphson step (max rel err 8.8e-4,
# validated on hw): y0 = bits(M - (v>>1)); r = -(C1 v y0^2 - C2) y0 / C3i
M_MAGIC = 0x5f3759df
RSQ_C1 = 0.4995951108461808
RSQ_C2 = 1.4987462744275555
RSQ_C3I = 1.0017264996621233

_CACHE = {}


def _build(use_beta, use_gamma):
    import concourse.bacc as bacc
    import concourse.tile as tile
    from concourse import mybir

    F32 = mybir.dt.float32
    F32R = mybir.dt.float32r
    BF16 = mybir.dt.bfloat16
    I32 = mybir.dt.int32
    AX = mybir.AxisListType.X
    OP = mybir.AluOpType
    AF = mybir.ActivationFunctionType

    nc = bacc.Bacc()
    d_xT = nc.declare_dram_parameter("xT", [64, ICHUNK], F32R, isOutput=False)
    d_ctxT = nc.declare_dram_parameter("ctxT", [64, N], F32R, isOutput=False)
    d_wqT = nc.declare_dram_parameter("wqT", [64, 64], F32R, isOutput=False)
    d_wkT = nc.declare_dram_parameter("wkT", [64, 64], F32R, isOutput=False)
    d_wvT = nc.declare_dram_parameter("wvT", [64, 64], F32R, isOutput=False)
    d_WD = nc.declare_dram_parameter("WD", [128, 128], F32, isOutput=False)
    d_beta = nc.declare_dram_parameter("beta", [1, 128], F32, isOutput=False)
    d_gtf = nc.declare_dram_parameter("gtf", [1, 64], F32, isOutput=False)
    d_gvf = nc.declare_dram_parameter("gvf", [1, 64], F32, isOutput=False)
    d_bvf = nc.declare_dram_parameter("bvf", [1, 64], F32, isOutput=False)
    d_mask = nc.declare_dram_parameter("mask", [128, 64], F32, isOutput=False)
    d_ones = nc.declare_dram_parameter("ones", [128, 1], F32, isOutput=False)
    d_out = nc.declare_dram_parameter("out", [NT * 128, 8], F32, isOutput=True)

    import concourse.bass as bass

    def bcast_ap(ap, levels):
        return bass.AP(tensor=ap.tensor, offset=ap.offset, ap=levels)

    with tile.TileContext(nc) as tc:
        with tc.tile_pool(name="statics", bufs=1) as st:
            xT = st.tile([64, ICHUNK], F32R)
            ctxT = st.tile([64, N], F32R)
            wqT = st.tile([64, 64], F32R)
            wkT = st.tile([64, 64], F32R)
            wvT = st.tile([64, 64], F32R)
            WD = st.tile([128, 128], F32)
            WDb = st.tile([128, 128], BF16)
            beta = st.tile([1, 128], F32)
            gtf = st.tile([128, 64], F32)
            gvf = st.tile([128, 64], F32)
            bvf = st.tile([128, 64], F32)
            mask = st.tile([128, 64], F32)
            ones = st.tile([128, 1], F32)
            for sb, dr in ((xT, d_xT), (ctxT, d_ctxT), (wqT, d_wqT),
                           (wkT, d_wkT), (wvT, d_wvT), (WD, d_WD),
                           (beta, d_beta), (mask, d_mask), (ones, d_ones)):
                nc.sync.dma_start(out=sb[:], in_=dr[:])
            for sb, dr in ((gtf, d_gtf), (gvf, d_gvf), (bvf, d_bvf)):
                nc.sync.dma_start(
                    out=sb[:], in_=bcast_ap(dr[:], [[0, 128], [1, 64]]))
            nc.vector.tensor_copy(out=WDb[:], in_=WD[:])


            qT = st.tile([64, ICHUNK], F32R)
            kT = st.tile([64, N], F32R)
            Vraw = st.tile([128, 1024], F32)
            Vn = st.tile([128, 1024], F32)
            Vng = st.tile([128, 1024], F32)
            BD = st.tile([64, NT * 128], F32R)
            vs_sb = st.tile([1, 1024], F32)
            Vsum = st.tile([1, 64], F32)

            # ---------------- prologue ----------------
            with tc.tile_pool(name="pps", bufs=1, space="PSUM") as pps:
                qps = pps.tile([64, ICHUNK], F32, tag="q")
                nc.tensor.matmul(qps[:], wqT[:], xT[:], start=True, stop=True)
                nc.scalar.copy(out=qT[:], in_=qps[:])
                for q4 in range(4):
                    kps = pps.tile([64, 512], F32, tag="k")
                    nc.tensor.matmul(kps[:], wkT[:],
                                     ctxT[:, q4 * 512:(q4 + 1) * 512],
                                     start=True, stop=True)
                    nc.scalar.copy(out=kT[:, q4 * 512:(q4 + 1) * 512], in_=kps[:])
                for c in range(16):
                    vps = pps.tile([128, 64], F32, tag="v")
                    nc.tensor.matmul(vps[:], ctxT[:, c * 128:(c + 1) * 128],
                                     wvT[:], start=True, stop=True)
                    nc.vector.tensor_copy(out=Vraw[:, c * 64:(c + 1) * 64],
                                          in_=vps[:])

                # per-head LayerNorm of v over d (groups of 8 in free dim)
                MU8 = st.tile([128, 128], F32)
                S2 = st.tile([128, 128], F32)
                Vsq = st.tile([128, 1024], F32)
                v4 = Vraw[:].rearrange("p (c h d) -> p c h d", h=8, d=8)
                nc.vector.tensor_reduce(out=MU8[:], in_=v4, axis=AX, op=OP.add)
                nc.vector.tensor_mul(out=Vsq[:], in0=Vraw[:], in1=Vraw[:])
                nc.vector.tensor_reduce(
                    out=S2[:], in_=Vsq[:].rearrange("p (c h d) -> p c h d", h=8, d=8),
                    axis=AX, op=OP.add)
                mu = st.tile([128, 128], F32)
                nc.vector.tensor_scalar_mul(out=mu[:], in0=MU8[:], scalar1=0.125)
                musq = st.tile([128, 128], F32)
                nc.vector.tensor_mul(out=musq[:], in0=mu[:], in1=mu[:])
                varv = st.tile([128, 128], F32)
                nc.vector.tensor_scalar_mul(out=varv[:], in0=S2[:],
                                            scalar1=0.125)
                nc.vector.tensor_sub(out=varv[:], in0=varv[:], in1=musq[:])
                nc.vector.tensor_scalar_add(out=varv[:], in0=varv[:],
                                            scalar1=float(EPS))
                lnv = st.tile([128, 128], F32)
                nc.scalar.activation(out=lnv[:], in_=varv[:], func=AF.Ln)
                rv = st.tile([128, 128], F32)
                nc.scalar.activation(out=rv[:], in_=lnv[:], func=AF.Exp,
                                     scale=-0.5)
                muv = mu[:].rearrange("p (c h) -> p c h", h=8)
                mub = bcast_ap(muv, [muv.ap[0], muv.ap[1], muv.ap[2], [0, 8]])
                rvv = rv[:].rearrange("p (c h) -> p c h", h=8)
                rvb = bcast_ap(rvv, [rvv.ap[0], rvv.ap[1], rvv.ap[2], [0, 8]])
                nc.vector.tensor_sub(out=v4, in0=v4, in1=mub)
                nc.vector.tensor_mul(out=v4, in0=v4, in1=rvb)
                v3 = Vraw[:].rearrange("p (c hd) -> p c hd", hd=64)
                gva = gvf[:]
                gvb = bcast_ap(gva, [gva.ap[0], [0, 16], [1, 64]])
                bva = bvf[:]
                bvb = bcast_ap(bva, [bva.ap[0], [0, 16], [1, 64]])
                nc.vector.tensor_mul(out=Vn[:].rearrange("p (c hd) -> p c hd", hd=64),
                                     in0=v3, in1=gvb)
                nc.vector.tensor_add(out=Vn[:].rearrange("p (c hd) -> p c hd", hd=64),
                                     in0=Vn[:].rearrange("p (c hd) -> p c hd", hd=64),
                                     in1=bvb)
                if use_beta:
                    for hf in range(2):
                        vsps = pps.tile([1, 512], F32, tag="vs")
                        nc.tensor.matmul(vsps[:], ones[:],
                                         Vn[:, hf * 512:(hf + 1) * 512],
                                         start=True, stop=True)
                        nc.vector.tensor_copy(
                            out=vs_sb[:, hf * 512:(hf + 1) * 512], in_=vsps[:])
                    vsv = vs_sb[:]
                    nc.vector.tensor_reduce(
                        out=Vsum[:],
                        in_=bcast_ap(vsv, [vsv.ap[0], [1, 64], [64, 16]]),
                        axis=AX, op=OP.add)
                if use_gamma:
                    gta = gtf[:]
                    gtb = bcast_ap(gta, [gta.ap[0], [0, 16], [1, 64]])
                    nc.vector.tensor_mul(
                        out=Vng[:].rearrange("p (c hd) -> p c hd", hd=64),
                        in0=Vn[:].rearrange("p (c hd) -> p c hd", hd=64), in1=gtb)
                    AVrhs = Vng
                else:
                    AVrhs = Vn
                Vng16 = st.tile([128, 1024], BF16)
                nc.vector.tensor_copy(out=Vng16[:], in_=AVrhs[:])
                AVrhs = Vng16

                # block-diagonal packed q: BD[(h,d), (t, h, i16)] = qT[(h,d), (t,i)]
                nc.vector.memset(BD[:].bitcast(mybir.dt.int32), 0)
                BD3 = BD[:].rearrange("p (t c) -> p t c", c=128)
                qT3 = qT[:].rearrange("p (t i) -> p t i", i=16)
                for h in range(8):
                    nc.sync.dma_start(
                        out=BD3[h * 8:(h + 1) * 8, :, h * 16:(h + 1) * 16],
                        in_=qT3[h * 8:(h + 1) * 8, :, :])

            # ---------------- main loop ----------------
            # tuned NR with C1 folded away: rr = (v*y0^2 - C2/C1)*y0,
            # r = rr * (-C1*C3); the (-C1*C3) lives in the host-side mask
            c2f = float(RSQ_C2 / RSQ_C1)
            # RR is scaled by c3p on write so the P multiply is a plain
            # all-bf16 tensor_tensor (2x perf mode; STT is 1x-only).
            with tc.tile_pool(name="sps", bufs=2, space="PSUM") as sps, \
                 tc.tile_pool(name="aps", bufs=2, space="PSUM") as aps, \
                 tc.tile_pool(name="avps", bufs=2, space="PSUM") as avps, \
                 tc.tile_pool(name="le", bufs=4) as le, \
                 tc.tile_pool(name="labf", bufs=6) as labf, \
                 tc.tile_pool(name="lsq", bufs=2) as lsq, \
                 tc.tile_pool(name="lt1", bufs=2) as lt1, \
                 tc.tile_pool(name="lp", bufs=2) as lp, \
                 tc.tile_pool(name="lrr", bufs=2) as lrr, \
                 tc.tile_pool(name="ltw", bufs=4) as ltw, \
                 tc.tile_pool(name="lsm", bufs=4) as lsm, \
                 tc.tile_pool(name="lout", bufs=3) as lout:
                S = {}   # per-tile live state
                PS = {}  # per-pair live state (SQ, RR)

                def stage_front(t):
                    # QK + exp + den for tile t
                    st_ = S[t] = {}
                    bd_t = BD[:, t * 128:(t + 1) * 128]
                    E = le.tile([128, N], BF16, tag="E")
                    st_["E"] = E
                    den4 = lsm.tile([128, 2], F32, tag="den4")
                    for q2 in range(2):
                        s_t = sps.tile([128, 1024], F32, tag="s")
                        for qh in range(2):
                            nc.tensor.matmul(
                                s_t[:, qh * 512:(qh + 1) * 512], bd_t,
                                kT[:, (q2 * 2 + qh) * 512:(q2 * 2 + qh + 1) * 512],
                                start=True, stop=True)
                        nc.scalar.activation(
                            out=E[:, q2 * 1024:(q2 + 1) * 1024], in_=s_t[:],
                            func=AF.Exp, accum_out=den4[:, q2:q2 + 1])
                    st_["den4"] = den4

                def stage_escale(t):
                    # den -> 1/den -> tw = WD/den (128 cols, 16x cheaper
                    # than scaling E; per-(h,i) so it precedes head mixing)
                    st_ = S[t]
                    den = lsm.tile([128, 1], F32, tag="den")
                    nc.vector.tensor_reduce(out=den[:], in_=st_["den4"],
                                            axis=AX, op=OP.add)
                    rden = lsm.tile([128, 1], F32, tag="rden")
                    nc.vector.reciprocal_approx_fast(out=rden[:], in_=den[:])
                    tw = ltw.tile([128, 128], BF16, tag="tw")
                    st_["tw"] = tw
                    rdb = rden[:]
                    nc.vector.tensor_mul(
                        out=tw[:], in0=WDb[:],
                        in1=bcast_ap(rdb, [rdb.ap[0], [0, 128]]))

                def stage_talk(t):
                    # talk matmuls + PSUM->SBUF bf16 copies + squares
                    st_ = S[t]
                    E = st_["E"]
                    Abf = labf.tile([128, N], BF16, tag="Abf")
                    st_["Abf"] = Abf
                    u, half = divmod(t, 2)
                    if half == 0:
                        SQ = lsq.tile([128, 2 * N], BF16, tag="SQ")
                        PS[u] = {"SQ": SQ}
                    else:
                        SQ = PS[u]["SQ"]
                    off = half * N
                    for b4 in range(4):
                        a_t = aps.tile([128, 512], F32, tag="a")
                        for cl in range(4):
                            c = b4 * 4 + cl
                            nc.tensor.matmul(a_t[:, cl * 128:(cl + 1) * 128],
                                             E[:, c * 128:(c + 1) * 128],
                                             st_["tw"][:], start=True,
                                             stop=True)
                        nc.scalar.copy(out=Abf[:, b4 * 512:(b4 + 1) * 512],
                                       in_=a_t[:])
                        if b4 in (1, 3):
                            h = b4 // 2
                            nc.gpsimd.tensor_mul(
                                out=SQ[:, off + h * 1024:off + (h + 1) * 1024],
                                in0=Abf[:, h * 1024:(h + 1) * 1024],
                                in1=Abf[:, h * 1024:(h + 1) * 1024])

                def stage_stats(u):
                    # pair-batched m2 tree + quake rsqrt for tiles 2u, 2u+1
                    ps = PS[u]
                    SQ = ps["SQ"]
                    s4 = SQ[:].rearrange("p (c x) -> p c x", x=128)
                    T1 = lt1.tile([128, 2048], BF16, tag="T1")
                    t4 = T1[:].rearrange("p (c x) -> p c x", x=64)
                    nc.vector.tensor_add(out=t4, in0=s4[:, :, 0:64],
                                         in1=s4[:, :, 64:128])
                    T2 = lt1.tile([128, 1024], BF16, tag="T2")
                    t24 = T2[:].rearrange("p (c x) -> p c x", x=32)
                    nc.vector.tensor_add(out=t24, in0=t4[:, :, 0:32],
                                         in1=t4[:, :, 32:64])
                    M2 = lsm.tile([128, 512], F32, tag="M2")
                    nc.vector.tensor_reduce(
                        out=M2[:],
                        in_=T2[:].rearrange("p (c two i) -> p c i two",
                                            two=2, i=16),
                        axis=AX, op=OP.add)
                    VP = lsm.tile([128, 512], F32, tag="VP")
                    nc.vector.tensor_scalar(out=VP[:], in0=M2[:],
                                            scalar1=0.125, scalar2=float(EPS),
                                            op0=OP.mult, op1=OP.add)
                    Y0 = lsm.tile([128, 512], F32, tag="Y0")
                    nc.vector.tensor_scalar(out=Y0[:].bitcast(I32),
                                            in0=VP[:].bitcast(I32),
                                            scalar1=1, scalar2=0xFFFFFFFF,
                                            op0=OP.logical_shift_right,
                                            op1=OP.bitwise_xor)
                    nc.vector.tensor_scalar(out=Y0[:].bitcast(I32),
                                            in0=Y0[:].bitcast(I32),
                                            scalar1=M_MAGIC + 1, scalar2=None,
                                            op0=OP.add)
                    TT = lsm.tile([128, 512], F32, tag="TT")
                    nc.vector.tensor_mul(out=TT[:], in0=Y0[:], in1=Y0[:])
                    UU = lsm.tile([128, 512], F32, tag="UU")
                    nc.vector.tensor_mul(out=UU[:], in0=TT[:], in1=VP[:])
                    # RR = (UU - c2f) * Y0; the c3p factor is folded into
                    # the host-side mask (and 1/c3p into beta)
                    RR = lrr.tile([128, 512], BF16, tag="RR")
                    ps["RR"] = RR
                    nc.vector.tensor_scalar_sub(out=UU[:], in0=UU[:],
                                                scalar1=c2f)
                    nc.vector.tensor_mul(out=RR[:], in0=UU[:], in1=Y0[:])

                def stage_out(t):
                    # P = RRE * Abf (bf16 2x), AV matmuls, extract h=g
                    st_ = S[t]
                    u, half = divmod(t, 2)
                    RR = PS[u]["RR"]
                    Abf = st_["Abf"]
                    P = lp.tile([128, N], BF16, tag="P")
                    for eng, lo in ((nc.gpsimd, 1), (nc.vector, 0)):
                        rr3 = RR[:, half * 256 + lo * 128:
                                 half * 256 + (lo + 1) * 128].rearrange(
                            "p (c i) -> p c i", i=16)
                        rb = bcast_ap(rr3, [rr3.ap[0], rr3.ap[1], [0, 8],
                                            rr3.ap[2]])
                        eng.tensor_mul(
                            out=P[:, lo * 1024:(lo + 1) * 1024].rearrange(
                                "p (c g i) -> p c g i", g=8, i=16),
                            in0=rb,
                            in1=Abf[:, lo * 1024:(lo + 1) * 1024].rearrange(
                                "p (c g i) -> p c g i", g=8, i=16))
                    av = avps.tile([128, 64], F32, tag="av")
                    for c in range(16):
                        nc.tensor.matmul(av[:], P[:, c * 128:(c + 1) * 128],
                                         AVrhs[:, c * 64:(c + 1) * 64],
                                         start=(c == 0),
                                         stop=(c == 15 and not use_beta))
                    if use_beta:
                        nc.tensor.matmul(av[:], beta[:], Vsum[:],
                                         start=False, stop=True)
                    EX = lout.tile([128, 64], F32, tag="EX")
                    nc.vector.tensor_mul(out=EX[:], in0=av[:], in1=mask[:])
                    RES = lout.tile([128, 8], F32, tag="RES")
                    nc.vector.tensor_reduce(
                        out=RES[:],
                        in_=EX[:].rearrange("p (h d) -> p d h", h=8),
                        axis=AX, op=OP.add)
                    nc.sync.dma_start(out=d_out[t * 128:(t + 1) * 128, :],
                                      in_=RES[:])
                    del S[t]
                    if half == 1:
                        del PS[u]

                # software-pipelined emission; every cross-engine edge
                # has >= 1 full iteration of slack:
                #   talk/copies/SQ run 2 iters after exp, stats 4, out 5
                for it in range(NT + 6):
                    if 0 <= it - 1 < NT:
                        stage_escale(it - 1)
                    if 0 <= it - 3 < NT:
                        stage_talk(it - 3)
                    if it < NT:
                        stage_front(it)
                    if 0 <= it - 5 < NT and (it - 5) % 2 == 1:
                        stage_stats((it - 5) // 2)
                    if 0 <= it - 6 < NT:
                        stage_out(it - 6)
    nc.compile()
    return nc


def _get_module(use_beta, use_gamma):
    key = (use_beta, use_gamma)
    if key not in _CACHE:
        _CACHE[key] = _build(use_beta, use_gamma)
    return _CACHE[key]


def kernel(x, context, Wq, Wkv, g_v, b_v, W_talk, g_t, b_t, **_unused):
    from concourse.bass_utils import run_bass_kernel_spmd

    x = np.asarray(x, np.float32)
    context = np.asarray(context, np.float32)
    Wq = np.asarray(Wq, np.float32)
    Wkv = np.asarray(Wkv, np.float32)
    g_v = np.asarray(g_v, np.float32)
    b_v = np.asarray(b_v, np.float32)
    W_talk = np.asarray(W_talk, np.float32)
    g_t = np.asarray(g_t, np.float32)
    b_t = np.asarray(b_t, np.float32)

    use_beta = bool(np.any(b_t != 0.0))
    use_gamma = bool(np.any(g_t != 1.0))
    nc = _get_module(use_beta, use_gamma)

    wqT = np.ascontiguousarray(Wq.T) * np.float32(DIM ** -0.5)
    wkT = np.ascontiguousarray(Wkv[:DIM, :].T)
    wvT = np.ascontiguousarray(Wkv[DIM:, :].T)
    Wc = W_talk - W_talk.mean(axis=0, keepdims=True)
    # WD[(h,i'), (g,i)] = Wc[g,h] iff i == i'
    WD = np.zeros((8, 16, 8, 16), np.float32)
    for i in range(16):
        WD[:, i, :, i] = Wc.T          # WD[h,i,g,i] = Wc[g,h]
    WD = np.ascontiguousarray(WD.reshape(128, 128))
    beta = np.ascontiguousarray(np.repeat(b_t, 16)[None, :]
                                / np.float32(-RSQ_C1 * RSQ_C3I))
    gtf = np.ascontiguousarray(np.repeat(g_t, 8)[None, :])
    gvf = np.ascontiguousarray(np.tile(g_v, 8)[None, :])
    bvf = np.ascontiguousarray(np.tile(b_v, 8)[None, :])
    # mask[(g,i), (h,d)] = c3p * (h == g); c3p folds the NR constant and
    # sign so the on-chip rsqrt chain skips a scaling op
    c3p = -RSQ_C1 * RSQ_C3I
    mask = np.zeros((8, 16, 8, 8), np.float32)
    for g in range(8):
        mask[g, :, g, :] = c3p
    mask = np.ascontiguousarray(mask.reshape(128, 64))
    ones = np.ones((128, 1), np.float32)

    in_maps = []
    for c in range(NCORES):
        b = c // 4
        i0 = (c % 4) * ICHUNK
        in_maps.append({
            "xT": np.ascontiguousarray(x[b, i0:i0 + ICHUNK, :].T),
            "ctxT": np.ascontiguousarray(context[b].T),
            "wqT": wqT, "wkT": wkT, "wvT": wvT, "WD": WD, "beta": beta,
            "gtf": gtf, "gvf": gvf, "bvf": bvf, "mask": mask, "ones": ones,
        })
    trace_dir = globals().get("TRACE_TMPDIR")
    if trace_dir:
        res = run_bass_kernel_spmd(nc, in_maps, list(range(NCORES)),
                                   trace=True, tmpdir=trace_dir)
        globals()["LAST_EXEC_NS"] = res.exec_time_ns
    else:
        res = run_bass_kernel_spmd(nc, in_maps, list(range(NCORES)))
    out = np.empty((B, 2048, DIM), np.float32)
    for c in range(NCORES):
        b = c // 4
        i0 = (c % 4) * ICHUNK
        o = res.results[c]["out"].reshape(NT, 8, 16, 8)   # [t, g, i, d]
        out[b, i0:i0 + ICHUNK, :] = (
            o.transpose(0, 2, 1, 3).reshape(ICHUNK, DIM))
    return out


# revision 30
# speedup vs baseline: 1.0983x; 1.0983x over previous
"""Trainium2 Bass kernel for nn_CrossAttention_56092272886201.

Talking-heads cross-attention, b=2, n=m=2048, dim=64, heads=8, dh=dv=8.
Sharding: 8 cores = (batch 2) x (query-chunks of 512). Each core is fully
independent (talking-heads mixes the heads axis, which stays on-core; the
query axis i is sharded), so no collectives are needed.

Per-core layout (i-tile = 16 queries x 8 heads = 128 partitions):
  S[(h,i), j]     QK^T via block-diagonal packed q, fp32r matmuls (1 cy/row)
  E = exp(S)      ACT, fused row-sum -> den (per (h,i) partition)
  tw = WD/den     one GPSIMD normalize_recip op, bf16 out (den is per-head,
                  so the softmax normalization must precede the head mixing)
  A^T[j,(i,g)]    talk matmul, lhsT=E-chunk, rhs=tw
  Abf             ACT copy of A PSUM->SBUF bf16
  SQ = Abf^2      GPSIMD, m2 = sum_g SQ via DVE windowed tensor_reduce
  r               DVE-only rsqrt of v = m2/8 + eps: quake seed (int
                  shift/xor/add) + one tuned Newton step, no ACT table swaps
  P = Abf*r*C3    one DVE STT op (all-bf16), C3 fixes NR sign/scale
  out             P @ V_ln + beta via K=1 matmul; mask+reduce extracts h=g
W_talk is centered host-side over g so the heads-LayerNorm mean is exactly 0.
"""

import numpy as np

DIM = 64
HEADS = 8
N = 2048
B = 2
NCORES = 8
ICHUNK = 512          # queries per core
NT = 32               # i-tiles per core (16 queries each)
EPS = 1e-5

# quake-III rsqrt with one tuned Newton-Raphson step (max rel err 8.8e-4,
# validated on hw): y0 = bits(M - (v>>1)); r = -(C1 v y0^2 - C2) y0 / C3i
M_MAGIC = 0x5f3759df
RSQ_C1 = 0.4995951108461808
RSQ_C2 = 1.4987462744275555
RSQ_C3I = 1.0017264996621233

_CACHE = {}


def _build(use_beta, use_gamma):
    import concourse.bacc as bacc
    import concourse.tile as tile
    from concourse import mybir

    F32 = mybir.dt.float32
    F32R = mybir.dt.float32r
    BF16 = mybir.dt.bfloat16
    I32 = mybir.dt.int32
    AX = mybir.AxisListType.X
    OP = mybir.AluOpType
    AF = mybir.ActivationFunctionType

    nc = bacc.Bacc()
    d_xT = nc.declare_dram_parameter("xT", [64, ICHUNK], F32R, isOutput=False)
    d_ctxT = nc.declare_dram_parameter("ctxT", [64, N], F32R, isOutput=False)
    d_wqT = nc.declare_dram_parameter("wqT", [64, 64], F32R, isOutput=False)
    d_wkT = nc.declare_dram_parameter("wkT", [64, 64], F32R, isOutput=False)
    d_wvT = nc.declare_dram_parameter("wvT", [64, 64], F32R, isOutput=False)
    d_WD = nc.declare_dram_parameter("WD", [128, 128], F32, isOutput=False)
    d_beta = nc.declare_dram_parameter("beta", [1, 128], F32, isOutput=False)
    d_gtf = nc.declare_dram_parameter("gtf", [1, 64], F32, isOutput=False)
    d_gvf = nc.declare_dram_parameter("gvf", [1, 64], F32, isOutput=False)
    d_bvf = nc.declare_dram_parameter("bvf", [1, 64], F32, isOutput=False)
    d_mask = nc.declare_dram_parameter("mask", [128, 64], F32, isOutput=False)
    d_ones = nc.declare_dram_parameter("ones", [128, 1], F32, isOutput=False)
    d_out = nc.declare_dram_parameter("out", [NT * 128, 8], F32, isOutput=True)

    import concourse.bass as bass

    def bcast_ap(ap, levels):
        return bass.AP(tensor=ap.tensor, offset=ap.offset, ap=levels)

    with tile.TileContext(nc) as tc:
        with tc.tile_pool(name="statics", bufs=1) as st:
            xT = st.tile([64, ICHUNK], F32R)
            ctxT = st.tile([64, N], F32R)
            wqT = st.tile([64, 64], F32R)
            wkT = st.tile([64, 64], F32R)
            wvT = st.tile([64, 64], F32R)
            WD = st.tile([128, 128], F32)
            WDb = st.tile([128, 128], BF16)
            beta = st.tile([1, 128], F32)
            gtf = st.tile([128, 64], F32)
            gvf = st.tile([128, 64], F32)
            bvf = st.tile([128, 64], F32)
            mask = st.tile([128, 64], F32)
            ones = st.tile([128, 1], F32)
            for sb, dr in ((xT, d_xT), (ctxT, d_ctxT), (wqT, d_wqT),
                           (wkT, d_wkT), (wvT, d_wvT), (WD, d_WD),
                           (beta, d_beta), (mask, d_mask), (ones, d_ones)):
                nc.sync.dma_start(out=sb[:], in_=dr[:])
            for sb, dr in ((gtf, d_gtf), (gvf, d_gvf), (bvf, d_bvf)):
                nc.sync.dma_start(
                    out=sb[:], in_=bcast_ap(dr[:], [[0, 128], [1, 64]]))
            nc.vector.tensor_copy(out=WDb[:], in_=WD[:])


            qT = st.tile([64, ICHUNK], F32R)
            kT = st.tile([64, N], F32R)
            Vraw = st.tile([128, 1024], F32)
            Vn = st.tile([128, 1024], F32)
            Vng = st.tile([128, 1024], F32)
            BD = st.tile([64, NT * 128], F32R)
            vs_sb = st.tile([1, 1024], F32)
            Vsum = st.tile([1, 64], F32)

            # ---------------- prologue ----------------
            with tc.tile_pool(name="pps", bufs=1, space="PSUM") as pps:
                qps = pps.tile([64, ICHUNK], F32, tag="q")
                nc.tensor.matmul(qps[:], wqT[:], xT[:], start=True, stop=True)
                nc.scalar.copy(out=qT[:], in_=qps[:])
                for q4 in range(4):
                    kps = pps.tile([64, 512], F32, tag="k")
                    nc.tensor.matmul(kps[:], wkT[:],
                                     ctxT[:, q4 * 512:(q4 + 1) * 512],
                                     start=True, stop=True)
                    nc.scalar.copy(out=kT[:, q4 * 512:(q4 + 1) * 512], in_=kps[:])
                for c in range(16):
                    vps = pps.tile([128, 64], F32, tag="v")
                    nc.tensor.matmul(vps[:], ctxT[:, c * 128:(c + 1) * 128],
                                     wvT[:], start=True, stop=True)
                    nc.vector.tensor_copy(out=Vraw[:, c * 64:(c + 1) * 64],
                                          in_=vps[:])

                # per-head LayerNorm of v over d (groups of 8 in free dim)
                MU8 = st.tile([128, 128], F32)
                S2 = st.tile([128, 128], F32)
                Vsq = st.tile([128, 1024], F32)
                v4 = Vraw[:].rearrange("p (c h d) -> p c h d", h=8, d=8)
                nc.vector.tensor_reduce(out=MU8[:], in_=v4, axis=AX, op=OP.add)
                nc.vector.tensor_mul(out=Vsq[:], in0=Vraw[:], in1=Vraw[:])
                nc.vector.tensor_reduce(
                    out=S2[:], in_=Vsq[:].rearrange("p (c h d) -> p c h d", h=8, d=8),
                    axis=AX, op=OP.add)
                mu = st.tile([128, 128], F32)
                nc.vector.tensor_scalar_mul(out=mu[:], in0=MU8[:], scalar1=0.125)
                musq = st.tile([128, 128], F32)
                nc.vector.tensor_mul(out=musq[:], in0=mu[:], in1=mu[:])
                varv = st.tile([128, 128], F32)
                nc.vector.tensor_scalar_mul(out=varv[:], in0=S2[:],
                                            scalar1=0.125)
                nc.vector.tensor_sub(out=varv[:], in0=varv[:], in1=musq[:])
                nc.vector.tensor_scalar_add(out=varv[:], in0=varv[:],
                                            scalar1=float(EPS))
                lnv = st.tile([128, 128], F32)
                nc.scalar.activation(out=lnv[:], in_=varv[:], func=AF.Ln)
                rv = st.tile([128, 128], F32)
                nc.scalar.activation(out=rv[:], in_=lnv[:], func=AF.Exp,
                                     scale=-0.5)
                muv = mu[:].rearrange("p (c h) -> p c h", h=8)
                mub = bcast_ap(muv, [muv.ap[0], muv.ap[1], muv.ap[2], [0, 8]])
                rvv = rv[:].rearrange("p (c h) -> p c h", h=8)
                rvb = bcast_ap(rvv, [rvv.ap[0], rvv.ap[1], rvv.ap[2], [0, 8]])
                nc.vector.tensor_sub(out=v4, in0=v4, in1=mub)
                nc.vector.tensor_mul(out=v4, in0=v4, in1=rvb)
                v3 = Vraw[:].rearrange("p (c hd) -> p c hd", hd=64)
                gva = gvf[:]
                gvb = bcast_ap(gva, [gva.ap[0], [0, 16], [1, 64]])
                bva = bvf[:]
                bvb = bcast_ap(bva, [bva.ap[0], [0, 16], [1, 64]])
                nc.vector.tensor_mul(out=Vn[:].rearrange("p (c hd) -> p c hd", hd=64),
                                     in0=v3, in1=gvb)
                nc.vector.tensor_add(out=Vn[:].rearrange("p (c hd) -> p c hd", hd=64),
                                     in0=Vn[:].rearrange("p (c hd) -> p c hd", hd=64),
                                     in1=bvb)
                if use_beta:
                    for hf in range(2):
                        vsps = pps.tile([1, 512], F32, tag="vs")
                        nc.tensor.matmul(vsps[:], ones[:],
                                         Vn[:, hf * 512:(hf + 1) * 512],
                                         start=True, stop=True)
                        nc.vector.tensor_copy(
                            out=vs_sb[:, hf * 512:(hf + 1) * 512], in_=vsps[:])
                    vsv = vs_sb[:]
                    nc.vector.tensor_reduce(
                        out=Vsum[:],
                        in_=bcast_ap(vsv, [vsv.ap[0], [1, 64], [64, 16]]),
                        axis=AX, op=OP.add)
                if use_gamma:
                    gta = gtf[:]
                    gtb = bcast_ap(gta, [gta.ap[0], [0, 16], [1, 64]])
                    nc.vector.tensor_mul(
                        out=Vng[:].rearrange("p (c hd) -> p c hd", hd=64),
                        in0=Vn[:].rearrange("p (c hd) -> p c hd", hd=64), in1=gtb)
                    AVrhs = Vng
                else:
                    AVrhs = Vn
                Vng16 = st.tile([128, 1024], BF16)
                nc.vector.tensor_copy(out=Vng16[:], in_=AVrhs[:])
                AVrhs = Vng16

                # block-diagonal packed q: BD[(h,d), (t, h, i16)] = qT[(h,d), (t,i)]
                nc.vector.memset(BD[:].bitcast(mybir.dt.int32), 0)
                BD3 = BD[:].rearrange("p (t c) -> p t c", c=128)
                qT3 = qT[:].rearrange("p (t i) -> p t i", i=16)
                for h in range(8):
                    nc.sync.dma_start(
                        out=BD3[h * 8:(h + 1) * 8, :, h * 16:(h + 1) * 16],
                        in_=qT3[h * 8:(h + 1) * 8, :, :])

            # ---------------- main loop ----------------
            # tuned NR with C1 folded away: rr = (v*y0^2 - C2/C1)*y0,
            # r = rr * (-C1*C3); the (-C1*C3) lives in the host-side mask
            c2f = float(RSQ_C2 / RSQ_C1)
            # RR is scaled by c3p on write so the P multiply is a plain
            # all-bf16 tensor_tensor (2x perf mode; STT is 1x-only).
            with tc.tile_pool(name="sps", bufs=2, space="PSUM") as sps, \
                 tc.tile_pool(name="aps", bufs=2, space="PSUM") as aps, \
                 tc.tile_pool(name="avps", bufs=2, space="PSUM") as avps, \
                 tc.tile_pool(name="le", bufs=4) as le, \
                 tc.tile_pool(name="labf", bufs=6) as labf, \
                 tc.tile_pool(name="lsq", bufs=2) as lsq, \
                 tc.tile_pool(name="lt1", bufs=2) as lt1, \
                 tc.tile_pool(name="lp", bufs=2) as lp, \
                 tc.tile_pool(name="lrr", bufs=2) as lrr, \
                 tc.tile_pool(name="ltw", bufs=4) as ltw, \
                 tc.tile_pool(name="lsm", bufs=4) as lsm, \
                 tc.tile_pool(name="lout", bufs=3) as lout:
                S = {}   # per-tile live state
                PS = {}  # per-pair live state (SQ, RR)

                def stage_front(t):
                    # QK + exp + den for tile t
                    st_ = S[t] = {}
                    bd_t = BD[:, t * 128:(t + 1) * 128]
                    E = le.tile([128, N], BF16, tag="E")
                    st_["E"] = E
                    den4 = lsm.tile([128, 2], F32, tag="den4")
                    for q2 in range(2):
                        s_t = sps.tile([128, 1024], F32, tag="s")
                        for qh in range(2):
                            nc.tensor.matmul(
                                s_t[:, qh * 512:(qh + 1) * 512], bd_t,
                                kT[:, (q2 * 2 + qh) * 512:(q2 * 2 + qh + 1) * 512],
                                start=True, stop=True)
                        nc.scalar.activation(
                            out=E[:, q2 * 1024:(q2 + 1) * 1024], in_=s_t[:],
                            func=AF.Exp, accum_out=den4[:, q2:q2 + 1])
                    st_["den4"] = den4

                def stage_escale(t):
                    # den -> 1/den -> tw = WD/den (128 cols, 16x cheaper
                    # than scaling E; per-(h,i) so it precedes head mixing)
                    st_ = S[t]
                    den = lsm.tile([128, 1], F32, tag="den")
                    nc.vector.tensor_reduce(out=den[:], in_=st_["den4"],
                                            axis=AX, op=OP.add)
                    rden = lsm.tile([128, 1], F32, tag="rden")
                    nc.vector.reciprocal_approx_fast(out=rden[:], in_=den[:])
                    tw = ltw.tile([128, 128], BF16, tag="tw")
                    st_["tw"] = tw
                    rdb = rden[:]
                    nc.vector.tensor_mul(
                        out=tw[:], in0=WDb[:],
                        in1=bcast_ap(rdb, [rdb.ap[0], [0, 128]]))

                def stage_talk(t):
                    # talk matmuls + PSUM->SBUF bf16 copies + squares
                    st_ = S[t]
                    E = st_["E"]
                    Abf = labf.tile([128, N], BF16, tag="Abf")
                    st_["Abf"] = Abf
                    u, half = divmod(t, 2)
                    if half == 0:
                        SQ = lsq.tile([128, 2 * N], BF16, tag="SQ")
                        PS[u] = {"SQ": SQ}
                    else:
                        SQ = PS[u]["SQ"]
                    off = half * N
                    for b4 in range(4):
                        a_t = aps.tile([128, 512], F32, tag="a")
                        for cl in range(4):
                            c = b4 * 4 + cl
                            nc.tensor.matmul(a_t[:, cl * 128:(cl + 1) * 128],
                                             E[:, c * 128:(c + 1) * 128],
                                             st_["tw"][:], start=True,
                                             stop=True)
                        nc.scalar.copy(out=Abf[:, b4 * 512:(b4 + 1) * 512],
                                       in_=a_t[:])
                        if b4 in (1, 3):
                            h = b4 // 2
                            nc.gpsimd.tensor_mul(
                                out=SQ[:, off + h * 1024:off + (h + 1) * 1024],
                                in0=Abf[:, h * 1024:(h + 1) * 1024],
                                in1=Abf[:, h * 1024:(h + 1) * 1024])

                def stage_stats(u):
                    # pair-batched m2 tree + quake rsqrt for tiles 2u, 2u+1
                    ps = PS[u]
                    SQ = ps["SQ"]
                    s4 = SQ[:].rearrange("p (c x) -> p c x", x=128)
                    T1 = lt1.tile([128, 2048], BF16, tag="T1")
                    t4 = T1[:].rearrange("p (c x) -> p c x", x=64)
                    nc.vector.tensor_add(out=t4, in0=s4[:, :, 0:64],
                                         in1=s4[:, :, 64:128])
                    T2 = lt1.tile([128, 1024], BF16, tag="T2")
                    t24 = T2[:].rearrange("p (c x) -> p c x", x=32)
                    nc.vector.tensor_add(out=t24, in0=t4[:, :, 0:32],
                                         in1=t4[:, :, 32:64])
                    M2 = lsm.tile([128, 512], F32, tag="M2")
                    nc.vector.tensor_reduce(
                        out=M2[:],
                        in_=T2[:].rearrange("p (c two i) -> p c i two",
                                            two=2, i=16),
                        axis=AX, op=OP.add)
                    VP = lsm.tile([128, 512], F32, tag="VP")
                    nc.vector.tensor_scalar(out=VP[:], in0=M2[:],
                                            scalar1=0.125, scalar2=float(EPS),
                                            op0=OP.mult, op1=OP.add)
                    Y0 = lsm.tile([128, 512], F32, tag="Y0")
                    nc.vector.tensor_scalar(out=Y0[:].bitcast(I32),
                                            in0=VP[:].bitcast(I32),
                                            scalar1=1, scalar2=0xFFFFFFFF,
                                            op0=OP.logical_shift_right,
                                            op1=OP.bitwise_xor)
                    nc.vector.tensor_scalar(out=Y0[:].bitcast(I32),
                                            in0=Y0[:].bitcast(I32),
                                            scalar1=M_MAGIC + 1, scalar2=None,
                                            op0=OP.add)
                    TT = lsm.tile([128, 512], F32, tag="TT")
                    nc.vector.tensor_mul(out=TT[:], in0=Y0[:], in1=Y0[:])
                    UU = lsm.tile([128, 512], F32, tag="UU")
                    nc.vector.tensor_mul(out=UU[:], in0=TT[:], in1=VP[:])
                    # RR = (UU - c2f) * Y0; the c3p factor is folded into
                    # the host-side mask (and 1/c3p into beta)
                    RR = lrr.tile([128, 512], BF16, tag="RR")
                    ps["RR"] = RR
                    nc.vector.tensor_scalar_sub(out=UU[:], in0=UU[:],
                                                scalar1=c2f)
                    nc.vector.tensor_mul(out=RR[:], in0=UU[:], in1=Y0[:])

                def stage_out(t):
                    # P = RRE * Abf (bf16 2x), AV matmuls, extract h=g
                    st_ = S[t]
                    u, half = divmod(t, 2)
                    RR = PS[u]["RR"]
                    Abf = st_["Abf"]
                    P = lp.tile([128, N], BF16, tag="P")
                    rr3 = RR[:, half * 256:(half + 1) * 256].rearrange(
                        "p (c i) -> p c i", i=16)
                    rb = bcast_ap(rr3, [rr3.ap[0], rr3.ap[1], [0, 8],
                                        rr3.ap[2]])
                    nc.vector.tensor_mul(
                        out=P[:].rearrange("p (c g i) -> p c g i", g=8, i=16),
                        in0=rb,
                        in1=Abf[:].rearrange("p (c g i) -> p c g i",
                                             g=8, i=16))
                    av = avps.tile([128, 64], F32, tag="av")
                    for c in range(16):
                        nc.tensor.matmul(av[:], P[:, c * 128:(c + 1) * 128],
                                         AVrhs[:, c * 64:(c + 1) * 64],
                                         start=(c == 0),
                                         stop=(c == 15 and not use_beta))
                    if use_beta:
                        nc.tensor.matmul(av[:], beta[:], Vsum[:],
                                         start=False, stop=True)
                    EX = lout.tile([128, 64], F32, tag="EX")
                    nc.vector.tensor_mul(out=EX[:], in0=av[:], in1=mask[:])
                    RES = lout.tile([128, 8], F32, tag="RES")
                    nc.vector.tensor_reduce(
                        out=RES[:],
                        in_=EX[:].rearrange("p (h d) -> p d h", h=8),
                        axis=AX, op=OP.add)
                    nc.sync.dma_start(out=d_out[t * 128:(t + 1) * 128, :],
                                      in_=RES[:])
                    del S[t]
                    if half == 1:
                        del PS[u]

                # software-pipelined emission; every cross-engine edge
                # has >= 1 full iteration of slack:
                #   talk/copies/SQ run 2 iters after exp, stats 4, out 5
                for it in range(NT + 6):
                    if 0 <= it - 1 < NT:
                        stage_escale(it - 1)
                    if 0 <= it - 3 < NT:
                        stage_talk(it - 3)
                    if it < NT:
                        stage_front(it)
                    if 0 <= it - 5 < NT and (it - 5) % 2 == 1:
                        stage_stats((it - 5) // 2)
                    if 0 <= it - 6 < NT:
                        stage_out(it - 6)
    nc.compile()
    return nc


def _get_module(use_beta, use_gamma):
    key = (use_beta, use_gamma)
    if key not in _CACHE:
        _CACHE[key] = _build(use_beta, use_gamma)
    return _CACHE[key]


def kernel(x, context, Wq, Wkv, g_v, b_v, W_talk, g_t, b_t, **_unused):
    from concourse.bass_utils import run_bass_kernel_spmd

    x = np.asarray(x, np.float32)
    context = np.asarray(context, np.float32)
    Wq = np.asarray(Wq, np.float32)
    Wkv = np.asarray(Wkv, np.float32)
    g_v = np.asarray(g_v, np.float32)
    b_v = np.asarray(b_v, np.float32)
    W_talk = np.asarray(W_talk, np.float32)
    g_t = np.asarray(g_t, np.float32)
    b_t = np.asarray(b_t, np.float32)

    use_beta = bool(np.any(b_t != 0.0))
    use_gamma = bool(np.any(g_t != 1.0))
    nc = _get_module(use_beta, use_gamma)

    wqT = np.ascontiguousarray(Wq.T) * np.float32(DIM ** -0.5)
    wkT = np.ascontiguousarray(Wkv[:DIM, :].T)
    wvT = np.ascontiguousarray(Wkv[DIM:, :].T)
    Wc = W_talk - W_talk.mean(axis=0, keepdims=True)
    # WD[(h,i'), (g,i)] = Wc[g,h] iff i == i'
    WD = np.zeros((8, 16, 8, 16), np.float32)
    for i in range(16):
        WD[:, i, :, i] = Wc.T          # WD[h,i,g,i] = Wc[g,h]
    WD = np.ascontiguousarray(WD.reshape(128, 128))
    beta = np.ascontiguousarray(np.repeat(b_t, 16)[None, :]
                                / np.float32(-RSQ_C1 * RSQ_C3I))
    gtf = np.ascontiguousarray(np.repeat(g_t, 8)[None, :])
    gvf = np.ascontiguousarray(np.tile(g_v, 8)[None, :])
    bvf = np.ascontiguousarray(np.tile(b_v, 8)[None, :])
    # mask[(g,i), (h,d)] = c3p * (h == g); c3p folds the NR constant and
    # sign so the on-chip rsqrt chain skips a scaling op
    c3p = -RSQ_C1 * RSQ_C3I
    mask = np.zeros((8, 16, 8, 8), np.float32)
    for g in range(8):
        mask[g, :, g, :] = c3p
    mask = np.ascontiguousarray(mask.reshape(128, 64))
    ones = np.ones((128, 1), np.float32)

    in_maps = []
    for c in range(NCORES):
        b = c // 4
        i0 = (c % 4) * ICHUNK
        in_maps.append({
            "xT": np.ascontiguousarray(x[b, i0:i0 + ICHUNK, :].T),
            "ctxT": np.ascontiguousarray(context[b].T),
            "wqT": wqT, "wkT": wkT, "wvT": wvT, "WD": WD, "beta": beta,
            "gtf": gtf, "gvf": gvf, "bvf": bvf, "mask": mask, "ones": ones,
        })
    trace_dir = globals().get("TRACE_TMPDIR")
    if trace_dir:
        res = run_bass_kernel_spmd(nc, in_maps, list(range(NCORES)),
                                   trace=True, tmpdir=trace_dir)
        globals()["LAST_EXEC_NS"] = res.exec_time_ns
    else:
        res = run_bass_kernel_spmd(nc, in_maps, list(range(NCORES)))
    out = np.empty((B, 2048, DIM), np.float32)
    for c in range(NCORES):
        b = c // 4
        i0 = (c % 4) * ICHUNK
        o = res.results[c]["out"].reshape(NT, 8, 16, 8)   # [t, g, i, d]
        out[b, i0:i0 + ICHUNK, :] = (
            o.transpose(0, 2, 1, 3).reshape(ICHUNK, DIM))
    return out


# revision 31
# speedup vs baseline: 1.1100x; 1.0107x over previous
"""Trainium2 Bass kernel for nn_CrossAttention_56092272886201.

Talking-heads cross-attention, b=2, n=m=2048, dim=64, heads=8, dh=dv=8.
Sharding: 8 cores = (batch 2) x (query-chunks of 512). Each core is fully
independent (talking-heads mixes the heads axis, which stays on-core; the
query axis i is sharded), so no collectives are needed.

Per-core layout (i-tile = 16 queries x 8 heads = 128 partitions):
  S[(h,i), j]     QK^T via block-diagonal packed q, fp32r matmuls (1 cy/row)
  E = exp(S)      ACT, fused row-sum -> den (per (h,i) partition)
  tw = WD/den     one GPSIMD normalize_recip op, bf16 out (den is per-head,
                  so the softmax normalization must precede the head mixing)
  A^T[j,(i,g)]    talk matmul, lhsT=E-chunk, rhs=tw
  Abf             ACT copy of A PSUM->SBUF bf16
  SQ = Abf^2      GPSIMD, m2 = sum_g SQ via DVE windowed tensor_reduce
  r               DVE-only rsqrt of v = m2/8 + eps: quake seed (int
                  shift/xor/add) + one tuned Newton step, no ACT table swaps
  P = Abf*r*C3    one DVE STT op (all-bf16), C3 fixes NR sign/scale
  out             P @ V_ln + beta via K=1 matmul; mask+reduce extracts h=g
W_talk is centered host-side over g so the heads-LayerNorm mean is exactly 0.
"""

import numpy as np

DIM = 64
HEADS = 8
N = 2048
B = 2
NCORES = 8
ICHUNK = 512          # queries per core
NT = 32               # i-tiles per core (16 queries each)
EPS = 1e-5

# quake-III rsqrt with one tuned Newton-Raphson step (max rel err 8.8e-4,
# validated on hw): y0 = bits(M - (v>>1)); r = -(C1 v y0^2 - C2) y0 / C3i
M_MAGIC = 0x5f3759df
RSQ_C1 = 0.4995951108461808
RSQ_C2 = 1.4987462744275555
RSQ_C3I = 1.0017264996621233

_CACHE = {}


def _build(use_beta, use_gamma):
    import concourse.bacc as bacc
    import concourse.tile as tile
    from concourse import mybir

    F32 = mybir.dt.float32
    F32R = mybir.dt.float32r
    BF16 = mybir.dt.bfloat16
    I32 = mybir.dt.int32
    AX = mybir.AxisListType.X
    OP = mybir.AluOpType
    AF = mybir.ActivationFunctionType

    nc = bacc.Bacc()
    d_xT = nc.declare_dram_parameter("xT", [64, ICHUNK], F32R, isOutput=False)
    d_ctxT = nc.declare_dram_parameter("ctxT", [64, N], F32R, isOutput=False)
    d_wqT = nc.declare_dram_parameter("wqT", [64, 64], F32R, isOutput=False)
    d_wkT = nc.declare_dram_parameter("wkT", [64, 64], F32R, isOutput=False)
    d_wvT = nc.declare_dram_parameter("wvT", [64, 64], F32R, isOutput=False)
    d_WD = nc.declare_dram_parameter("WD", [128, 128], F32, isOutput=False)
    d_beta = nc.declare_dram_parameter("beta", [1, 128], F32, isOutput=False)
    d_gtf = nc.declare_dram_parameter("gtf", [1, 64], F32, isOutput=False)
    d_gvf = nc.declare_dram_parameter("gvf", [1, 64], F32, isOutput=False)
    d_bvf = nc.declare_dram_parameter("bvf", [1, 64], F32, isOutput=False)
    d_mask = nc.declare_dram_parameter("mask", [128, 64], F32, isOutput=False)
    d_ones = nc.declare_dram_parameter("ones", [128, 1], F32, isOutput=False)
    d_out = nc.declare_dram_parameter("out", [NT * 128, 8], F32, isOutput=True)

    import concourse.bass as bass

    def bcast_ap(ap, levels):
        return bass.AP(tensor=ap.tensor, offset=ap.offset, ap=levels)

    with tile.TileContext(nc) as tc:
        with tc.tile_pool(name="statics", bufs=1) as st:
            xT = st.tile([64, ICHUNK], F32R)
            ctxT = st.tile([64, N], F32R)
            wqT = st.tile([64, 64], F32R)
            wkT = st.tile([64, 64], F32R)
            wvT = st.tile([64, 64], F32R)
            WD = st.tile([128, 128], F32)
            WDb = st.tile([128, 128], BF16)
            beta = st.tile([1, 128], F32)
            gtf = st.tile([128, 64], F32)
            gvf = st.tile([128, 64], F32)
            bvf = st.tile([128, 64], F32)
            mask = st.tile([128, 64], F32)
            ones = st.tile([128, 1], F32)
            for sb, dr in ((xT, d_xT), (ctxT, d_ctxT), (wqT, d_wqT),
                           (wkT, d_wkT), (wvT, d_wvT), (WD, d_WD),
                           (beta, d_beta), (mask, d_mask), (ones, d_ones)):
                nc.sync.dma_start(out=sb[:], in_=dr[:])
            for sb, dr in ((gtf, d_gtf), (gvf, d_gvf), (bvf, d_bvf)):
                nc.sync.dma_start(
                    out=sb[:], in_=bcast_ap(dr[:], [[0, 128], [1, 64]]))
            nc.vector.tensor_copy(out=WDb[:], in_=WD[:])


            qT = st.tile([64, ICHUNK], F32R)
            kT = st.tile([64, N], F32R)
            Vraw = st.tile([128, 1024], F32)
            Vn = st.tile([128, 1024], F32)
            Vng = st.tile([128, 1024], F32)
            BD = st.tile([64, NT * 128], F32R)
            vs_sb = st.tile([1, 1024], F32)
            Vsum = st.tile([1, 64], F32)

            # ---------------- prologue ----------------
            with tc.tile_pool(name="pps", bufs=1, space="PSUM") as pps:
                qps = pps.tile([64, ICHUNK], F32, tag="q")
                nc.tensor.matmul(qps[:], wqT[:], xT[:], start=True, stop=True)
                nc.scalar.copy(out=qT[:], in_=qps[:])
                for q4 in range(4):
                    kps = pps.tile([64, 512], F32, tag="k")
                    nc.tensor.matmul(kps[:], wkT[:],
                                     ctxT[:, q4 * 512:(q4 + 1) * 512],
                                     start=True, stop=True)
                    nc.scalar.copy(out=kT[:, q4 * 512:(q4 + 1) * 512], in_=kps[:])
                for c in range(16):
                    vps = pps.tile([128, 64], F32, tag="v")
                    nc.tensor.matmul(vps[:], ctxT[:, c * 128:(c + 1) * 128],
                                     wvT[:], start=True, stop=True)
                    nc.vector.tensor_copy(out=Vraw[:, c * 64:(c + 1) * 64],
                                          in_=vps[:])

                # per-head LayerNorm of v over d (groups of 8 in free dim)
                MU8 = st.tile([128, 128], F32)
                S2 = st.tile([128, 128], F32)
                Vsq = st.tile([128, 1024], F32)
                v4 = Vraw[:].rearrange("p (c h d) -> p c h d", h=8, d=8)
                nc.vector.tensor_reduce(out=MU8[:], in_=v4, axis=AX, op=OP.add)
                nc.vector.tensor_mul(out=Vsq[:], in0=Vraw[:], in1=Vraw[:])
                nc.vector.tensor_reduce(
                    out=S2[:], in_=Vsq[:].rearrange("p (c h d) -> p c h d", h=8, d=8),
                    axis=AX, op=OP.add)
                mu = st.tile([128, 128], F32)
                nc.vector.tensor_scalar_mul(out=mu[:], in0=MU8[:], scalar1=0.125)
                musq = st.tile([128, 128], F32)
                nc.vector.tensor_mul(out=musq[:], in0=mu[:], in1=mu[:])
                varv = st.tile([128, 128], F32)
                nc.vector.tensor_scalar_mul(out=varv[:], in0=S2[:],
                                            scalar1=0.125)
                nc.vector.tensor_sub(out=varv[:], in0=varv[:], in1=musq[:])
                nc.vector.tensor_scalar_add(out=varv[:], in0=varv[:],
                                            scalar1=float(EPS))
                lnv = st.tile([128, 128], F32)
                nc.scalar.activation(out=lnv[:], in_=varv[:], func=AF.Ln)
                rv = st.tile([128, 128], F32)
                nc.scalar.activation(out=rv[:], in_=lnv[:], func=AF.Exp,
                                     scale=-0.5)
                muv = mu[:].rearrange("p (c h) -> p c h", h=8)
                mub = bcast_ap(muv, [muv.ap[0], muv.ap[1], muv.ap[2], [0, 8]])
                rvv = rv[:].rearrange("p (c h) -> p c h", h=8)
                rvb = bcast_ap(rvv, [rvv.ap[0], rvv.ap[1], rvv.ap[2], [0, 8]])
                nc.vector.tensor_sub(out=v4, in0=v4, in1=mub)
                nc.vector.tensor_mul(out=v4, in0=v4, in1=rvb)
                v3 = Vraw[:].rearrange("p (c hd) -> p c hd", hd=64)
                gva = gvf[:]
                gvb = bcast_ap(gva, [gva.ap[0], [0, 16], [1, 64]])
                bva = bvf[:]
                bvb = bcast_ap(bva, [bva.ap[0], [0, 16], [1, 64]])
                nc.vector.tensor_mul(out=Vn[:].rearrange("p (c hd) -> p c hd", hd=64),
                                     in0=v3, in1=gvb)
                nc.vector.tensor_add(out=Vn[:].rearrange("p (c hd) -> p c hd", hd=64),
                                     in0=Vn[:].rearrange("p (c hd) -> p c hd", hd=64),
                                     in1=bvb)
                if use_beta:
                    for hf in range(2):
                        vsps = pps.tile([1, 512], F32, tag="vs")
                        nc.tensor.matmul(vsps[:], ones[:],
                                         Vn[:, hf * 512:(hf + 1) * 512],
                                         start=True, stop=True)
                        nc.vector.tensor_copy(
                            out=vs_sb[:, hf * 512:(hf + 1) * 512], in_=vsps[:])
                    vsv = vs_sb[:]
                    nc.vector.tensor_reduce(
                        out=Vsum[:],
                        in_=bcast_ap(vsv, [vsv.ap[0], [1, 64], [64, 16]]),
                        axis=AX, op=OP.add)
                if use_gamma:
                    gta = gtf[:]
                    gtb = bcast_ap(gta, [gta.ap[0], [0, 16], [1, 64]])
                    nc.vector.tensor_mul(
                        out=Vng[:].rearrange("p (c hd) -> p c hd", hd=64),
                        in0=Vn[:].rearrange("p (c hd) -> p c hd", hd=64), in1=gtb)
                    AVrhs = Vng
                else:
                    AVrhs = Vn
                Vng16 = st.tile([128, 1024], BF16)
                nc.vector.tensor_copy(out=Vng16[:], in_=AVrhs[:])
                AVrhs = Vng16

                # block-diagonal packed q: BD[(h,d), (t, h, i16)] = qT[(h,d), (t,i)]
                nc.vector.memset(BD[:].bitcast(mybir.dt.int32), 0)
                BD3 = BD[:].rearrange("p (t c) -> p t c", c=128)
                qT3 = qT[:].rearrange("p (t i) -> p t i", i=16)
                for h in range(8):
                    nc.sync.dma_start(
                        out=BD3[h * 8:(h + 1) * 8, :, h * 16:(h + 1) * 16],
                        in_=qT3[h * 8:(h + 1) * 8, :, :])

            # ---------------- main loop ----------------
            # tuned NR with C1 folded away: rr = (v*y0^2 - C2/C1)*y0,
            # r = rr * (-C1*C3); the (-C1*C3) lives in the host-side mask
            c2f = float(RSQ_C2 / RSQ_C1)
            # RR is scaled by c3p on write so the P multiply is a plain
            # all-bf16 tensor_tensor (2x perf mode; STT is 1x-only).
            with tc.tile_pool(name="sps", bufs=2, space="PSUM") as sps, \
                 tc.tile_pool(name="aps", bufs=2, space="PSUM") as aps, \
                 tc.tile_pool(name="avps", bufs=2, space="PSUM") as avps, \
                 tc.tile_pool(name="le", bufs=4) as le, \
                 tc.tile_pool(name="labf", bufs=6) as labf, \
                 tc.tile_pool(name="lsq", bufs=2) as lsq, \
                 tc.tile_pool(name="lt1", bufs=2) as lt1, \
                 tc.tile_pool(name="lp", bufs=2) as lp, \
                 tc.tile_pool(name="lrr", bufs=2) as lrr, \
                 tc.tile_pool(name="ltw", bufs=4) as ltw, \
                 tc.tile_pool(name="lsm", bufs=4) as lsm, \
                 tc.tile_pool(name="lout", bufs=3) as lout:
                S = {}   # per-tile live state
                PS = {}  # per-pair live state (SQ, RR)

                def stage_front(t):
                    # QK + exp + den for tile t
                    st_ = S[t] = {}
                    bd_t = BD[:, t * 128:(t + 1) * 128]
                    E = le.tile([128, N], BF16, tag="E")
                    st_["E"] = E
                    den4 = lsm.tile([128, 2], F32, tag="den4")
                    for q2 in range(2):
                        s_t = sps.tile([128, 1024], F32, tag="s")
                        for qh in range(2):
                            nc.tensor.matmul(
                                s_t[:, qh * 512:(qh + 1) * 512], bd_t,
                                kT[:, (q2 * 2 + qh) * 512:(q2 * 2 + qh + 1) * 512],
                                start=True, stop=True)
                        nc.scalar.activation(
                            out=E[:, q2 * 1024:(q2 + 1) * 1024], in_=s_t[:],
                            func=AF.Exp, accum_out=den4[:, q2:q2 + 1])
                    st_["den4"] = den4

                def stage_escale(t):
                    # den -> 1/den -> tw = WD/den (128 cols, 16x cheaper
                    # than scaling E; per-(h,i) so it precedes head mixing)
                    st_ = S[t]
                    den = lsm.tile([128, 1], F32, tag="den")
                    nc.vector.tensor_reduce(out=den[:], in_=st_["den4"],
                                            axis=AX, op=OP.add)
                    rden = lsm.tile([128, 1], F32, tag="rden")
                    nc.vector.reciprocal_approx_fast(out=rden[:], in_=den[:])
                    tw = ltw.tile([128, 128], BF16, tag="tw")
                    st_["tw"] = tw
                    rdb = rden[:]
                    nc.vector.tensor_mul(
                        out=tw[:], in0=WDb[:],
                        in1=bcast_ap(rdb, [rdb.ap[0], [0, 128]]))

                def stage_talk(t):
                    # talk matmuls + PSUM->SBUF bf16 copies + squares
                    st_ = S[t]
                    E = st_["E"]
                    Abf = labf.tile([128, N], BF16, tag="Abf")
                    st_["Abf"] = Abf
                    u, half = divmod(t, 2)
                    if half == 0:
                        SQ = lsq.tile([128, 2 * N], BF16, tag="SQ")
                        PS[u] = {"SQ": SQ}
                    else:
                        SQ = PS[u]["SQ"]
                    off = half * N
                    for b4 in range(4):
                        a_t = aps.tile([128, 512], F32, tag="a")
                        for cl in range(4):
                            c = b4 * 4 + cl
                            nc.tensor.matmul(a_t[:, cl * 128:(cl + 1) * 128],
                                             E[:, c * 128:(c + 1) * 128],
                                             st_["tw"][:], start=True,
                                             stop=True)
                        nc.scalar.copy(out=Abf[:, b4 * 512:(b4 + 1) * 512],
                                       in_=a_t[:])
                        if b4 in (1, 3):
                            h = b4 // 2
                            nc.gpsimd.tensor_mul(
                                out=SQ[:, off + h * 1024:off + (h + 1) * 1024],
                                in0=Abf[:, h * 1024:(h + 1) * 1024],
                                in1=Abf[:, h * 1024:(h + 1) * 1024])

                def stage_stats(u):
                    # pair-batched m2 tree + quake rsqrt for tiles 2u, 2u+1
                    ps = PS[u]
                    SQ = ps["SQ"]
                    s4 = SQ[:].rearrange("p (c x) -> p c x", x=128)
                    T1 = lt1.tile([128, 2048], BF16, tag="T1")
                    t4 = T1[:].rearrange("p (c x) -> p c x", x=64)
                    nc.vector.tensor_add(out=t4, in0=s4[:, :, 0:64],
                                         in1=s4[:, :, 64:128])
                    T2 = lt1.tile([128, 1024], BF16, tag="T2")
                    t24 = T2[:].rearrange("p (c x) -> p c x", x=32)
                    nc.vector.tensor_add(out=t24, in0=t4[:, :, 0:32],
                                         in1=t4[:, :, 32:64])
                    M2 = lsm.tile([128, 512], F32, tag="M2")
                    nc.vector.tensor_reduce(
                        out=M2[:],
                        in_=T2[:].rearrange("p (c two i) -> p c i two",
                                            two=2, i=16),
                        axis=AX, op=OP.add)
                    VP = lsm.tile([128, 512], F32, tag="VP")
                    nc.vector.tensor_scalar(out=VP[:], in0=M2[:],
                                            scalar1=0.125, scalar2=float(EPS),
                                            op0=OP.mult, op1=OP.add)
                    Y0 = lsm.tile([128, 512], F32, tag="Y0")
                    nc.vector.tensor_scalar(out=Y0[:].bitcast(I32),
                                            in0=VP[:].bitcast(I32),
                                            scalar1=1, scalar2=0xFFFFFFFF,
                                            op0=OP.logical_shift_right,
                                            op1=OP.bitwise_xor)
                    nc.vector.tensor_scalar(out=Y0[:].bitcast(I32),
                                            in0=Y0[:].bitcast(I32),
                                            scalar1=M_MAGIC + 1, scalar2=None,
                                            op0=OP.add)
                    TT = lsm.tile([128, 512], F32, tag="TT")
                    nc.vector.tensor_mul(out=TT[:], in0=Y0[:], in1=Y0[:])
                    UU = lsm.tile([128, 512], F32, tag="UU")
                    nc.vector.tensor_mul(out=UU[:], in0=TT[:], in1=VP[:])
                    # RR = (UU - c2f) * Y0; the c3p factor is folded into
                    # the host-side mask (and 1/c3p into beta)
                    RR = lrr.tile([128, 512], BF16, tag="RR")
                    ps["RR"] = RR
                    nc.vector.tensor_scalar_sub(out=UU[:], in0=UU[:],
                                                scalar1=c2f)
                    nc.vector.tensor_mul(out=RR[:], in0=UU[:], in1=Y0[:])

                def stage_out(t):
                    # P = RRE * Abf (bf16 2x), AV matmuls, extract h=g
                    st_ = S[t]
                    u, half = divmod(t, 2)
                    RR = PS[u]["RR"]
                    Abf = st_["Abf"]
                    P = lp.tile([128, N], BF16, tag="P")
                    rr3 = RR[:, half * 256:(half + 1) * 256].rearrange(
                        "p (c i) -> p c i", i=16)
                    rb = bcast_ap(rr3, [rr3.ap[0], rr3.ap[1], [0, 8],
                                        rr3.ap[2]])
                    nc.vector.tensor_mul(
                        out=P[:].rearrange("p (c g i) -> p c g i", g=8, i=16),
                        in0=rb,
                        in1=Abf[:].rearrange("p (c g i) -> p c g i",
                                             g=8, i=16))
                    av = avps.tile([128, 64], F32, tag="av")
                    for c in range(16):
                        nc.tensor.matmul(av[:], P[:, c * 128:(c + 1) * 128],
                                         AVrhs[:, c * 64:(c + 1) * 64],
                                         start=(c == 0),
                                         stop=(c == 15 and not use_beta))
                    if use_beta:
                        nc.tensor.matmul(av[:], beta[:], Vsum[:],
                                         start=False, stop=True)
                    EX = lout.tile([128, 64], F32, tag="EX")
                    nc.vector.tensor_mul(out=EX[:], in0=av[:], in1=mask[:])
                    RES = lout.tile([128, 8], F32, tag="RES")
                    nc.vector.tensor_reduce(
                        out=RES[:],
                        in_=EX[:].rearrange("p (h d) -> p d h", h=8),
                        axis=AX, op=OP.add)
                    nc.sync.dma_start(out=d_out[t * 128:(t + 1) * 128, :],
                                      in_=RES[:])
                    del S[t]
                    if half == 1:
                        del PS[u]

                # software-pipelined emission; every cross-engine edge
                # has >= 1 full iteration of slack:
                #   talk/copies/SQ run 2 iters after exp, stats 4, out 5
                for it in range(NT + 5):
                    if 0 <= it - 2 < NT:
                        stage_talk(it - 2)
                    if it < NT:
                        stage_front(it)
                    if 0 <= it - 4 < NT and (it - 4) % 2 == 1:
                        stage_stats((it - 4) // 2)
                    if 0 <= it - 5 < NT:
                        stage_out(it - 5)
                    if it < NT:
                        stage_escale(it)
    nc.compile()
    return nc


def _get_module(use_beta, use_gamma):
    key = (use_beta, use_gamma)
    if key not in _CACHE:
        _CACHE[key] = _build(use_beta, use_gamma)
    return _CACHE[key]


def kernel(x, context, Wq, Wkv, g_v, b_v, W_talk, g_t, b_t, **_unused):
    from concourse.bass_utils import run_bass_kernel_spmd

    x = np.asarray(x, np.float32)
    context = np.asarray(context, np.float32)
    Wq = np.asarray(Wq, np.float32)
    Wkv = np.asarray(Wkv, np.float32)
    g_v = np.asarray(g_v, np.float32)
    b_v = np.asarray(b_v, np.float32)
    W_talk = np.asarray(W_talk, np.float32)
    g_t = np.asarray(g_t, np.float32)
    b_t = np.asarray(b_t, np.float32)

    use_beta = bool(np.any(b_t != 0.0))
    use_gamma = bool(np.any(g_t != 1.0))
    nc = _get_module(use_beta, use_gamma)

    wqT = np.ascontiguousarray(Wq.T) * np.float32(DIM ** -0.5)
    wkT = np.ascontiguousarray(Wkv[:DIM, :].T)
    wvT = np.ascontiguousarray(Wkv[DIM:, :].T)
    Wc = W_talk - W_talk.mean(axis=0, keepdims=True)
    # WD[(h,i'), (g,i)] = Wc[g,h] iff i == i'
    WD = np.zeros((8, 16, 8, 16), np.float32)
    for i in range(16):
        WD[:, i, :, i] = Wc.T          # WD[h,i,g,i] = Wc[g,h]
    WD = np.ascontiguousarray(WD.reshape(128, 128))
    beta = np.ascontiguousarray(np.repeat(b_t, 16)[None, :]
                                / np.float32(-RSQ_C1 * RSQ_C3I))
    gtf = np.ascontiguousarray(np.repeat(g_t, 8)[None, :])
    gvf = np.ascontiguousarray(np.tile(g_v, 8)[None, :])
    bvf = np.ascontiguousarray(np.tile(b_v, 8)[None, :])
    # mask[(g,i), (h,d)] = c3p * (h == g); c3p folds the NR constant and
    # sign so the on-chip rsqrt chain skips a scaling op
    c3p = -RSQ_C1 * RSQ_C3I
    mask = np.zeros((8, 16, 8, 8), np.float32)
    for g in range(8):
        mask[g, :, g, :] = c3p
    mask = np.ascontiguousarray(mask.reshape(128, 64))
    ones = np.ones((128, 1), np.float32)

    in_maps = []
    for c in range(NCORES):
        b = c // 4
        i0 = (c % 4) * ICHUNK
        in_maps.append({
            "xT": np.ascontiguousarray(x[b, i0:i0 + ICHUNK, :].T),
            "ctxT": np.ascontiguousarray(context[b].T),
            "wqT": wqT, "wkT": wkT, "wvT": wvT, "WD": WD, "beta": beta,
            "gtf": gtf, "gvf": gvf, "bvf": bvf, "mask": mask, "ones": ones,
        })
    trace_dir = globals().get("TRACE_TMPDIR")
    if trace_dir:
        res = run_bass_kernel_spmd(nc, in_maps, list(range(NCORES)),
                                   trace=True, tmpdir=trace_dir)
        globals()["LAST_EXEC_NS"] = res.exec_time_ns
    else:
        res = run_bass_kernel_spmd(nc, in_maps, list(range(NCORES)))
    out = np.empty((B, 2048, DIM), np.float32)
    for c in range(NCORES):
        b = c // 4
        i0 = (c % 4) * ICHUNK
        o = res.results[c]["out"].reshape(NT, 8, 16, 8)   # [t, g, i, d]
        out[b, i0:i0 + ICHUNK, :] = (
            o.transpose(0, 2, 1, 3).reshape(ICHUNK, DIM))
    return out


# revision 32
# speedup vs baseline: 1.1137x; 1.0033x over previous
"""Trainium2 Bass kernel for nn_CrossAttention_56092272886201.

Talking-heads cross-attention, b=2, n=m=2048, dim=64, heads=8, dh=dv=8.
Sharding: 8 cores = (batch 2) x (query-chunks of 512). Each core is fully
independent (talking-heads mixes the heads axis, which stays on-core; the
query axis i is sharded), so no collectives are needed.

Per-core layout (i-tile = 16 queries x 8 heads = 128 partitions):
  S[(h,i), j]     QK^T via block-diagonal packed q, fp32r matmuls (1 cy/row
                  at N=512 vs 4 for fp32 -- the single biggest win)
  E = exp(S)      ACT, fused row-sum -> den (per (h,i) partition)
  tw = WD*rden    DVE (rden via reciprocal_approx_fast); den is per-head so
                  softmax normalization must precede the head mixing.
                  (gpsimd normalize_recip would thrash the Q7 ucode library
                  against the gpsimd squares: ~7us reload per switch)
  A^T[j,(g,i)]    talk matmul, lhsT=E-chunk, rhs=tw
  Abf             ACT copy of A PSUM->SBUF bf16
  SQ = Abf^2      GPSIMD (only SBUF->SBUF streams can live there)
  m2 = sum_g SQ   DVE bf16 tree adds, pair-batched across 2 tiles (x64/x32
                  strided halves hit the 2x perf mode; the x16 tail uses
                  tensor_reduce)
  r               DVE-only rsqrt of v = m2/8 + eps: quake seed (int
                  shift/xor/add) + one tuned Newton step => no ACT table
                  swaps (Ln/Exp alternation costs 2 x 1.5us per tile)
  P = Abf*r       one DVE bf16 tensor_tensor (g-broadcast on a middle AP
                  level keeps the 2x mode); the NR constant c3p is folded
                  into the host-side mask (and 1/c3p into beta)
  out             P @ V_ln + beta via K=1 matmul; mask+reduce extracts h=g
W_talk is centered host-side over g so the heads-LayerNorm mean is exactly 0.
The main loop is software-pipelined (emission staggered talk@-2/stats@-4/
out@-5) so each engine's in-order queue never head-of-line blocks on the
per-tile serial chain. 491us baseline -> ~312us.
"""

import numpy as np

DIM = 64
HEADS = 8
N = 2048
B = 2
NCORES = 8
ICHUNK = 512          # queries per core
NT = 32               # i-tiles per core (16 queries each)
EPS = 1e-5

# quake-III rsqrt with one tuned Newton-Raphson step (max rel err 8.8e-4,
# validated on hw): y0 = bits(M - (v>>1)); r = -(C1 v y0^2 - C2) y0 / C3i
M_MAGIC = 0x5f3759df
RSQ_C1 = 0.4995951108461808
RSQ_C2 = 1.4987462744275555
RSQ_C3I = 1.0017264996621233

_CACHE = {}


def _build(use_beta, use_gamma):
    import concourse.bacc as bacc
    import concourse.tile as tile
    from concourse import mybir

    F32 = mybir.dt.float32
    F32R = mybir.dt.float32r
    BF16 = mybir.dt.bfloat16
    I32 = mybir.dt.int32
    AX = mybir.AxisListType.X
    OP = mybir.AluOpType
    AF = mybir.ActivationFunctionType

    nc = bacc.Bacc()
    d_xT = nc.declare_dram_parameter("xT", [64, ICHUNK], F32R, isOutput=False)
    d_ctxT = nc.declare_dram_parameter("ctxT", [64, N], F32R, isOutput=False)
    d_wqT = nc.declare_dram_parameter("wqT", [64, 64], F32R, isOutput=False)
    d_wkT = nc.declare_dram_parameter("wkT", [64, 64], F32R, isOutput=False)
    d_wvT = nc.declare_dram_parameter("wvT", [64, 64], F32R, isOutput=False)
    d_WD = nc.declare_dram_parameter("WD", [128, 128], F32, isOutput=False)
    d_beta = nc.declare_dram_parameter("beta", [1, 128], F32, isOutput=False)
    d_gtf = nc.declare_dram_parameter("gtf", [1, 64], F32, isOutput=False)
    d_gvf = nc.declare_dram_parameter("gvf", [1, 64], F32, isOutput=False)
    d_bvf = nc.declare_dram_parameter("bvf", [1, 64], F32, isOutput=False)
    d_mask = nc.declare_dram_parameter("mask", [128, 64], F32, isOutput=False)
    d_ones = nc.declare_dram_parameter("ones", [128, 1], F32, isOutput=False)
    d_out = nc.declare_dram_parameter("out", [NT * 128, 8], F32, isOutput=True)

    import concourse.bass as bass

    def bcast_ap(ap, levels):
        return bass.AP(tensor=ap.tensor, offset=ap.offset, ap=levels)

    with tile.TileContext(nc) as tc:
        with tc.tile_pool(name="statics", bufs=1) as st:
            xT = st.tile([64, ICHUNK], F32R)
            ctxT = st.tile([64, N], F32R)
            wqT = st.tile([64, 64], F32R)
            wkT = st.tile([64, 64], F32R)
            wvT = st.tile([64, 64], F32R)
            WD = st.tile([128, 128], F32)
            WDb = st.tile([128, 128], BF16)
            beta = st.tile([1, 128], F32)
            gtf = st.tile([128, 64], F32)
            gvf = st.tile([128, 64], F32)
            bvf = st.tile([128, 64], F32)
            mask = st.tile([128, 64], F32)
            ones = st.tile([128, 1], F32)
            for sb, dr in ((xT, d_xT), (ctxT, d_ctxT), (wqT, d_wqT),
                           (wkT, d_wkT), (wvT, d_wvT), (WD, d_WD),
                           (beta, d_beta), (mask, d_mask), (ones, d_ones)):
                nc.sync.dma_start(out=sb[:], in_=dr[:])
            for sb, dr in ((gtf, d_gtf), (gvf, d_gvf), (bvf, d_bvf)):
                nc.sync.dma_start(
                    out=sb[:], in_=bcast_ap(dr[:], [[0, 128], [1, 64]]))
            nc.vector.tensor_copy(out=WDb[:], in_=WD[:])


            qT = st.tile([64, ICHUNK], F32R)
            kT = st.tile([64, N], F32R)
            Vraw = st.tile([128, 1024], F32)
            Vn = st.tile([128, 1024], F32)
            Vng = st.tile([128, 1024], F32)
            BD = st.tile([64, NT * 128], F32R)
            vs_sb = st.tile([1, 1024], F32)
            Vsum = st.tile([1, 64], F32)

            # ---------------- prologue ----------------
            with tc.tile_pool(name="pps", bufs=1, space="PSUM") as pps:
                qps = pps.tile([64, ICHUNK], F32, tag="q")
                nc.tensor.matmul(qps[:], wqT[:], xT[:], start=True, stop=True)
                nc.scalar.copy(out=qT[:], in_=qps[:])
                for q4 in range(4):
                    kps = pps.tile([64, 512], F32, tag="k")
                    nc.tensor.matmul(kps[:], wkT[:],
                                     ctxT[:, q4 * 512:(q4 + 1) * 512],
                                     start=True, stop=True)
                    nc.scalar.copy(out=kT[:, q4 * 512:(q4 + 1) * 512], in_=kps[:])
                for c in range(16):
                    vps = pps.tile([128, 64], F32, tag="v")
                    nc.tensor.matmul(vps[:], ctxT[:, c * 128:(c + 1) * 128],
                                     wvT[:], start=True, stop=True)
                    nc.vector.tensor_copy(out=Vraw[:, c * 64:(c + 1) * 64],
                                          in_=vps[:])

                # per-head LayerNorm of v over d (groups of 8 in free dim)
                MU8 = st.tile([128, 128], F32)
                S2 = st.tile([128, 128], F32)
                Vsq = st.tile([128, 1024], F32)
                v4 = Vraw[:].rearrange("p (c h d) -> p c h d", h=8, d=8)
                nc.vector.tensor_reduce(out=MU8[:], in_=v4, axis=AX, op=OP.add)
                nc.vector.tensor_mul(out=Vsq[:], in0=Vraw[:], in1=Vraw[:])
                nc.vector.tensor_reduce(
                    out=S2[:], in_=Vsq[:].rearrange("p (c h d) -> p c h d", h=8, d=8),
                    axis=AX, op=OP.add)
                mu = st.tile([128, 128], F32)
                nc.vector.tensor_scalar_mul(out=mu[:], in0=MU8[:], scalar1=0.125)
                musq = st.tile([128, 128], F32)
                nc.vector.tensor_mul(out=musq[:], in0=mu[:], in1=mu[:])
                varv = st.tile([128, 128], F32)
                nc.vector.tensor_scalar_mul(out=varv[:], in0=S2[:],
                                            scalar1=0.125)
                nc.vector.tensor_sub(out=varv[:], in0=varv[:], in1=musq[:])
                nc.vector.tensor_scalar_add(out=varv[:], in0=varv[:],
                                            scalar1=float(EPS))
                lnv = st.tile([128, 128], F32)
                nc.scalar.activation(out=lnv[:], in_=varv[:], func=AF.Ln)
                rv = st.tile([128, 128], F32)
                nc.scalar.activation(out=rv[:], in_=lnv[:], func=AF.Exp,
                                     scale=-0.5)
                muv = mu[:].rearrange("p (c h) -> p c h", h=8)
                mub = bcast_ap(muv, [muv.ap[0], muv.ap[1], muv.ap[2], [0, 8]])
                rvv = rv[:].rearrange("p (c h) -> p c h", h=8)
                rvb = bcast_ap(rvv, [rvv.ap[0], rvv.ap[1], rvv.ap[2], [0, 8]])
                nc.vector.tensor_sub(out=v4, in0=v4, in1=mub)
                nc.vector.tensor_mul(out=v4, in0=v4, in1=rvb)
                v3 = Vraw[:].rearrange("p (c hd) -> p c hd", hd=64)
                gva = gvf[:]
                gvb = bcast_ap(gva, [gva.ap[0], [0, 16], [1, 64]])
                bva = bvf[:]
                bvb = bcast_ap(bva, [bva.ap[0], [0, 16], [1, 64]])
                nc.vector.tensor_mul(out=Vn[:].rearrange("p (c hd) -> p c hd", hd=64),
                                     in0=v3, in1=gvb)
                nc.vector.tensor_add(out=Vn[:].rearrange("p (c hd) -> p c hd", hd=64),
                                     in0=Vn[:].rearrange("p (c hd) -> p c hd", hd=64),
                                     in1=bvb)
                if use_beta:
                    for hf in range(2):
                        vsps = pps.tile([1, 512], F32, tag="vs")
                        nc.tensor.matmul(vsps[:], ones[:],
                                         Vn[:, hf * 512:(hf + 1) * 512],
                                         start=True, stop=True)
                        nc.vector.tensor_copy(
                            out=vs_sb[:, hf * 512:(hf + 1) * 512], in_=vsps[:])
                    vsv = vs_sb[:]
                    nc.vector.tensor_reduce(
                        out=Vsum[:],
                        in_=bcast_ap(vsv, [vsv.ap[0], [1, 64], [64, 16]]),
                        axis=AX, op=OP.add)
                if use_gamma:
                    gta = gtf[:]
                    gtb = bcast_ap(gta, [gta.ap[0], [0, 16], [1, 64]])
                    nc.vector.tensor_mul(
                        out=Vng[:].rearrange("p (c hd) -> p c hd", hd=64),
                        in0=Vn[:].rearrange("p (c hd) -> p c hd", hd=64), in1=gtb)
                    AVrhs = Vng
                else:
                    AVrhs = Vn
                Vng16 = st.tile([128, 1024], BF16)
                nc.vector.tensor_copy(out=Vng16[:], in_=AVrhs[:])
                AVrhs = Vng16

                # block-diagonal packed q: BD[(h,d), (t, h, i16)] = qT[(h,d), (t,i)]
                nc.vector.memset(BD[:].bitcast(mybir.dt.int32), 0)
                BD3 = BD[:].rearrange("p (t c) -> p t c", c=128)
                qT3 = qT[:].rearrange("p (t i) -> p t i", i=16)
                for h in range(8):
                    nc.sync.dma_start(
                        out=BD3[h * 8:(h + 1) * 8, :, h * 16:(h + 1) * 16],
                        in_=qT3[h * 8:(h + 1) * 8, :, :])

            # ---------------- main loop ----------------
            # tuned NR with C1 folded away: rr = (v*y0^2 - C2/C1)*y0,
            # r = rr * (-C1*C3); the (-C1*C3) lives in the host-side mask
            c2f = float(RSQ_C2 / RSQ_C1)
            # RR is scaled by c3p on write so the P multiply is a plain
            # all-bf16 tensor_tensor (2x perf mode; STT is 1x-only).
            with tc.tile_pool(name="sps", bufs=2, space="PSUM") as sps, \
                 tc.tile_pool(name="aps", bufs=2, space="PSUM") as aps, \
                 tc.tile_pool(name="avps", bufs=2, space="PSUM") as avps, \
                 tc.tile_pool(name="le", bufs=4) as le, \
                 tc.tile_pool(name="labf", bufs=6) as labf, \
                 tc.tile_pool(name="lsq", bufs=2) as lsq, \
                 tc.tile_pool(name="lt1", bufs=2) as lt1, \
                 tc.tile_pool(name="lp", bufs=2) as lp, \
                 tc.tile_pool(name="lrr", bufs=2) as lrr, \
                 tc.tile_pool(name="ltw", bufs=4) as ltw, \
                 tc.tile_pool(name="lsm", bufs=4) as lsm, \
                 tc.tile_pool(name="lout", bufs=3) as lout:
                S = {}   # per-tile live state
                PS = {}  # per-pair live state (SQ, RR)

                def stage_front(t):
                    # QK + exp + den for tile t
                    st_ = S[t] = {}
                    bd_t = BD[:, t * 128:(t + 1) * 128]
                    E = le.tile([128, N], BF16, tag="E")
                    st_["E"] = E
                    den4 = lsm.tile([128, 2], F32, tag="den4")
                    for q2 in range(2):
                        s_t = sps.tile([128, 1024], F32, tag="s")
                        for qh in range(2):
                            nc.tensor.matmul(
                                s_t[:, qh * 512:(qh + 1) * 512], bd_t,
                                kT[:, (q2 * 2 + qh) * 512:(q2 * 2 + qh + 1) * 512],
                                start=True, stop=True)
                        nc.scalar.activation(
                            out=E[:, q2 * 1024:(q2 + 1) * 1024], in_=s_t[:],
                            func=AF.Exp, accum_out=den4[:, q2:q2 + 1])
                    st_["den4"] = den4

                def stage_escale(t):
                    # den -> 1/den -> tw = WD/den (128 cols, 16x cheaper
                    # than scaling E; per-(h,i) so it precedes head mixing)
                    st_ = S[t]
                    den = lsm.tile([128, 1], F32, tag="den")
                    nc.vector.tensor_reduce(out=den[:], in_=st_["den4"],
                                            axis=AX, op=OP.add)
                    rden = lsm.tile([128, 1], F32, tag="rden")
                    nc.vector.reciprocal_approx_fast(out=rden[:], in_=den[:])
                    tw = ltw.tile([128, 128], BF16, tag="tw")
                    st_["tw"] = tw
                    rdb = rden[:]
                    nc.vector.tensor_mul(
                        out=tw[:], in0=WDb[:],
                        in1=bcast_ap(rdb, [rdb.ap[0], [0, 128]]))

                def stage_talk(t):
                    # talk matmuls + PSUM->SBUF bf16 copies + squares
                    st_ = S[t]
                    E = st_["E"]
                    Abf = labf.tile([128, N], BF16, tag="Abf")
                    st_["Abf"] = Abf
                    u, half = divmod(t, 2)
                    if half == 0:
                        SQ = lsq.tile([128, 2 * N], BF16, tag="SQ")
                        PS[u] = {"SQ": SQ}
                    else:
                        SQ = PS[u]["SQ"]
                    off = half * N
                    for b4 in range(4):
                        a_t = aps.tile([128, 512], F32, tag="a")
                        for cl in range(4):
                            c = b4 * 4 + cl
                            nc.tensor.matmul(a_t[:, cl * 128:(cl + 1) * 128],
                                             E[:, c * 128:(c + 1) * 128],
                                             st_["tw"][:], start=True,
                                             stop=True)
                        nc.scalar.copy(out=Abf[:, b4 * 512:(b4 + 1) * 512],
                                       in_=a_t[:])
                        if b4 in (1, 3):
                            h = b4 // 2
                            nc.gpsimd.tensor_mul(
                                out=SQ[:, off + h * 1024:off + (h + 1) * 1024],
                                in0=Abf[:, h * 1024:(h + 1) * 1024],
                                in1=Abf[:, h * 1024:(h + 1) * 1024])

                def stage_stats(u):
                    # pair-batched m2 tree + quake rsqrt for tiles 2u, 2u+1
                    ps = PS[u]
                    SQ = ps["SQ"]
                    s4 = SQ[:].rearrange("p (c x) -> p c x", x=128)
                    T1 = lt1.tile([128, 2048], BF16, tag="T1")
                    t4 = T1[:].rearrange("p (c x) -> p c x", x=64)
                    nc.vector.tensor_add(out=t4, in0=s4[:, :, 0:64],
                                         in1=s4[:, :, 64:128])
                    T2 = lt1.tile([128, 1024], BF16, tag="T2")
                    t24 = T2[:].rearrange("p (c x) -> p c x", x=32)
                    nc.vector.tensor_add(out=t24, in0=t4[:, :, 0:32],
                                         in1=t4[:, :, 32:64])
                    M2 = lsm.tile([128, 512], F32, tag="M2")
                    nc.vector.tensor_reduce(
                        out=M2[:],
                        in_=T2[:].rearrange("p (c two i) -> p c i two",
                                            two=2, i=16),
                        axis=AX, op=OP.add)
                    VP = lsm.tile([128, 512], F32, tag="VP")
                    nc.vector.tensor_scalar(out=VP[:], in0=M2[:],
                                            scalar1=0.125, scalar2=float(EPS),
                                            op0=OP.mult, op1=OP.add)
                    Y0 = lsm.tile([128, 512], F32, tag="Y0")
                    nc.vector.tensor_scalar(out=Y0[:].bitcast(I32),
                                            in0=VP[:].bitcast(I32),
                                            scalar1=1, scalar2=0xFFFFFFFF,
                                            op0=OP.logical_shift_right,
                                            op1=OP.bitwise_xor)
                    nc.vector.tensor_scalar(out=Y0[:].bitcast(I32),
                                            in0=Y0[:].bitcast(I32),
                                            scalar1=M_MAGIC + 1, scalar2=None,
                                            op0=OP.add)
                    TT = lsm.tile([128, 512], F32, tag="TT")
                    nc.vector.tensor_mul(out=TT[:], in0=Y0[:], in1=Y0[:])
                    UU = lsm.tile([128, 512], F32, tag="UU")
                    nc.vector.tensor_mul(out=UU[:], in0=TT[:], in1=VP[:])
                    # RR = (UU - c2f) * Y0; the c3p factor is folded into
                    # the host-side mask (and 1/c3p into beta)
                    RR = lrr.tile([128, 512], BF16, tag="RR")
                    ps["RR"] = RR
                    nc.vector.tensor_scalar_sub(out=UU[:], in0=UU[:],
                                                scalar1=c2f)
                    nc.vector.tensor_mul(out=RR[:], in0=UU[:], in1=Y0[:])

                def stage_out(t):
                    # P = RRE * Abf (bf16 2x), AV matmuls, extract h=g
                    st_ = S[t]
                    u, half = divmod(t, 2)
                    RR = PS[u]["RR"]
                    Abf = st_["Abf"]
                    P = lp.tile([128, N], BF16, tag="P")
                    rr3 = RR[:, half * 256:(half + 1) * 256].rearrange(
                        "p (c i) -> p c i", i=16)
                    rb = bcast_ap(rr3, [rr3.ap[0], rr3.ap[1], [0, 8],
                                        rr3.ap[2]])
                    nc.vector.tensor_mul(
                        out=P[:].rearrange("p (c g i) -> p c g i", g=8, i=16),
                        in0=rb,
                        in1=Abf[:].rearrange("p (c g i) -> p c g i",
                                             g=8, i=16))
                    av = avps.tile([128, 64], F32, tag="av")
                    for c in range(16):
                        nc.tensor.matmul(av[:], P[:, c * 128:(c + 1) * 128],
                                         AVrhs[:, c * 64:(c + 1) * 64],
                                         start=(c == 0),
                                         stop=(c == 15 and not use_beta))
                    if use_beta:
                        nc.tensor.matmul(av[:], beta[:], Vsum[:],
                                         start=False, stop=True)
                    EX = lout.tile([128, 64], F32, tag="EX")
                    nc.vector.tensor_mul(out=EX[:], in0=av[:], in1=mask[:])
                    RES = lout.tile([128, 8], F32, tag="RES")
                    nc.vector.tensor_reduce(
                        out=RES[:],
                        in_=EX[:].rearrange("p (h d) -> p d h", h=8),
                        axis=AX, op=OP.add)
                    nc.sync.dma_start(out=d_out[t * 128:(t + 1) * 128, :],
                                      in_=RES[:])
                    del S[t]
                    if half == 1:
                        del PS[u]

                # software-pipelined emission; every cross-engine edge
                # has >= 1 full iteration of slack:
                #   talk/copies/SQ run 2 iters after exp, stats 4, out 5
                for it in range(NT + 5):
                    if 0 <= it - 2 < NT:
                        stage_talk(it - 2)
                    if it < NT:
                        stage_front(it)
                    if 0 <= it - 4 < NT and (it - 4) % 2 == 1:
                        stage_stats((it - 4) // 2)
                    if 0 <= it - 5 < NT:
                        stage_out(it - 5)
                    if it < NT:
                        stage_escale(it)
    nc.compile()
    return nc


def _get_module(use_beta, use_gamma):
    key = (use_beta, use_gamma)
    if key not in _CACHE:
        _CACHE[key] = _build(use_beta, use_gamma)
    return _CACHE[key]


def kernel(x, context, Wq, Wkv, g_v, b_v, W_talk, g_t, b_t, **_unused):
    from concourse.bass_utils import run_bass_kernel_spmd

    x = np.asarray(x, np.float32)
    context = np.asarray(context, np.float32)
    Wq = np.asarray(Wq, np.float32)
    Wkv = np.asarray(Wkv, np.float32)
    g_v = np.asarray(g_v, np.float32)
    b_v = np.asarray(b_v, np.float32)
    W_talk = np.asarray(W_talk, np.float32)
    g_t = np.asarray(g_t, np.float32)
    b_t = np.asarray(b_t, np.float32)

    use_beta = bool(np.any(b_t != 0.0))
    use_gamma = bool(np.any(g_t != 1.0))
    nc = _get_module(use_beta, use_gamma)

    wqT = np.ascontiguousarray(Wq.T) * np.float32(DIM ** -0.5)
    wkT = np.ascontiguousarray(Wkv[:DIM, :].T)
    wvT = np.ascontiguousarray(Wkv[DIM:, :].T)
    Wc = W_talk - W_talk.mean(axis=0, keepdims=True)
    # WD[(h,i'), (g,i)] = Wc[g,h] iff i == i'
    WD = np.zeros((8, 16, 8, 16), np.float32)
    for i in range(16):
        WD[:, i, :, i] = Wc.T          # WD[h,i,g,i] = Wc[g,h]
    WD = np.ascontiguousarray(WD.reshape(128, 128))
    beta = np.ascontiguousarray(np.repeat(b_t, 16)[None, :]
                                / np.float32(-RSQ_C1 * RSQ_C3I))
    gtf = np.ascontiguousarray(np.repeat(g_t, 8)[None, :])
    gvf = np.ascontiguousarray(np.tile(g_v, 8)[None, :])
    bvf = np.ascontiguousarray(np.tile(b_v, 8)[None, :])
    # mask[(g,i), (h,d)] = c3p * (h == g); c3p folds the NR constant and
    # sign so the on-chip rsqrt chain skips a scaling op
    c3p = -RSQ_C1 * RSQ_C3I
    mask = np.zeros((8, 16, 8, 8), np.float32)
    for g in range(8):
        mask[g, :, g, :] = c3p
    mask = np.ascontiguousarray(mask.reshape(128, 64))
    ones = np.ones((128, 1), np.float32)

    in_maps = []
    for c in range(NCORES):
        b = c // 4
        i0 = (c % 4) * ICHUNK
        in_maps.append({
            "xT": np.ascontiguousarray(x[b, i0:i0 + ICHUNK, :].T),
            "ctxT": np.ascontiguousarray(context[b].T),
            "wqT": wqT, "wkT": wkT, "wvT": wvT, "WD": WD, "beta": beta,
            "gtf": gtf, "gvf": gvf, "bvf": bvf, "mask": mask, "ones": ones,
        })
    trace_dir = globals().get("TRACE_TMPDIR")
    if trace_dir:
        res = run_bass_kernel_spmd(nc, in_maps, list(range(NCORES)),
                                   trace=True, tmpdir=trace_dir)
        globals()["LAST_EXEC_NS"] = res.exec_time_ns
    else:
        res = run_bass_kernel_spmd(nc, in_maps, list(range(NCORES)))
    out = np.empty((B, 2048, DIM), np.float32)
    for c in range(NCORES):
        b = c // 4
        i0 = (c % 4) * ICHUNK
        o = res.results[c]["out"].reshape(NT, 8, 16, 8)   # [t, g, i, d]
        out[b, i0:i0 + ICHUNK, :] = (
            o.transpose(0, 2, 1, 3).reshape(ICHUNK, DIM))
    return out
